# revision 24
# baseline (speedup 1.0000x reference)
"""Trainium2 Bass kernel for nn_MultiHeadAttention_62371515073076.

Math (per batch b, faithful to the reference's quirky softmax over the QUERY axis):
  q/k/v = einsum('nc,chd->nhd', x, W{q,k,v})
  s[i,j,h] = q[i,h,:].k[j,h,:] / 8
  p = softmax over i  (query axis!)
  attnw[i,h] = sum_j p[i,j,h]
             = sum_j exp(s[i,j,h]) / Z[j,h],   Z[j,h] = sum_i exp(s[i,j,h])
  out = einsum('ihd,ohd->io', v * attnw, Wout)

Sharding: batch 8 -> one batch per NeuronCore (data parallel), weights replicated.

Per-core layout (all fp16 activations, fp32 psum/accum):
  - Host pre-transposes x -> xt (C,N) and Wout -> wot (HD,O); wq/wk/wv fed
    natural (C,HD) concatenated = the lhsT layout the PE wants.
  - QKV projections produce transposed Q^T/K^T/V^T [hd, i] tiles.
  - Scores computed transposed, S^T [j, i], per head, K=64 row-packed two
    heads per PE pass (partitions 0-63 / 64-127).
  - exp via ScalarE reading the score PSUM directly (HW-measured: ScalarE is
    1x rate regardless of dtype/space, so any SBUF bounce just wastes DVE).
    Z[j] row-sums are split between ScalarE accum_out (+282ns/tile on ACT)
    and gpsimd tensor_reduce on the otherwise-idle Pool engine.
  - attnw colsum matmuls INTERLEAVE with the score loop (lag 1 jt) so the PE
    never idles >3.4us -- HAM re-throttles the PE clock 2.4->1.2 GHz after
    ~3.4us idle, which was worth ~40us/iter. lhsT = 1/Z column broadcast via
    a step-0 AP (two heads col-packed, accumulated over j in PSUM).
  - Next head-pair's projections are spread across the score loop so the PE
    always has independent work queued.
  - applied^T = V^T * attnw_bcast (DVE), then output projection back to
    natural [i, o] layout and DMA out.
"""
import os
import numpy as np
from contextlib import ExitStack

import concourse.bass as bass
import concourse.mybir as mybir
import concourse.tile as tile
from concourse import bacc
from concourse.vector_clock import ScopedClock
from concourse.bass_utils import run_bass_kernel_spmd
import bass_rust

N_CORES = 8
B, N, C, H, D, O = 8, 1024, 256, 8, 64, 256
HD = H * D  # 512
FP32 = mybir.dt.float32
F16 = mybir.dt.float16
EXP = mybir.ActivationFunctionType.Exp

# Of every 32 per-jt Z row-sum pairs, this many use ScalarE accum_out; the
# rest run as one DVE free-axis reduce per jt. Balances ACT vs DVE.
ZACC_NUM = int(os.environ.get("ZACC_NUM", "10"))

_MAXW = 1  # max sync waits this toolchain's walrus accepts per instruction


class _TC(tile.TileContext):
    """TileContext that splits semaphore waits one-per-instruction.

    The walrus build in this toolchain rejects any instruction carrying more
    than one sync wait ("Too many sync wait commands"), while Tile's
    add_semaphores attaches all needed waits to the consuming instruction.
    Engines execute in order, so moving excess waits onto same-engine NOPs
    emitted immediately before the instruction is semantically identical.
    """

    def _commit_instruction(self, inst, lazy_reg_writes: bool = True):
        si = inst.sync_info
        if (
            si is not None
            and si.on_wait
            and len(si.on_wait) > _MAXW
            and inst.engine != mybir.EngineType.Unassigned
        ):
            waits = list(si.on_wait)
            inst.sync_info = bass_rust.SyncInfo(
                on_wait=waits[-_MAXW:], on_update=list(si.on_update or [])
            )
            for i in range(0, len(waits) - _MAXW, _MAXW):
                nop = self.nc.engines[inst.engine].nop(nofuse=True, hint="waitsplit")
                nop.ins.sync_info = bass_rust.SyncInfo(
                    on_wait=waits[i : i + _MAXW], on_update=[]
                )
        return super()._commit_instruction(inst, lazy_reg_writes)

    def _drain_and_barrier(self, tick_clock, wait_clock):
        probe = self.nc.sync.drain()
        wait_clock.add_sem_waits(
            probe.ins, ScopedClock({None: tick_clock.global_clock})
        )
        si = probe.ins.sync_info
        waits = list(si.on_wait or []) if si is not None else []
        if len(waits) > 1:
            probe.ins.sync_info = bass_rust.SyncInfo(
                on_wait=waits[:1], on_update=list(si.on_update or [])
            )
            for i in range(1, len(waits)):
                d = self.nc.sync.drain()
                d.ins.sync_info = bass_rust.SyncInfo(
                    on_wait=waits[i : i + 1], on_update=[]
                )
        self.nc.all_engine_barrier()
        assert self.sems is not None
        popped = self.nc._tile_sem_poison_stack.pop()
        assert popped is self._sem_poison
        self.nc.clear_and_free_semaphores(list(self.sems.allocated().values()))
        self.nc.all_engine_barrier()


def _bcast64(col_ap):
    """[P,1] AP -> [P,64] AP reading the same element 64x (free step 0)."""
    return bass.AP(col_ap.tensor, col_ap.offset, [list(col_ap.ap[0]), [0, 64]])


def _split2(ap, inner):
    """[P,2*inner] AP -> [P,2,inner] AP (for per-half free-axis reduces)."""
    return bass.AP(ap.tensor, ap.offset, [list(ap.ap[0]), [inner, 2], [1, inner]])


def _emit_body(tc, xt, wqkv, wot, out):
    nc = tc.nc
    with ExitStack() as ctx:
        wpool = ctx.enter_context(tc.tile_pool(name="w", bufs=1))
        qkvpool = ctx.enter_context(tc.tile_pool(name="qkv", bufs=1))
        gpool = ctx.enter_context(tc.tile_pool(name="g", bufs=2))
        izpool = ctx.enter_context(tc.tile_pool(name="iz", bufs=2))
        zpool = ctx.enter_context(tc.tile_pool(name="z", bufs=2))
        obpool = ctx.enter_context(tc.tile_pool(name="ob", bufs=2))

        # fine-grained input loads, ordered by first use: ic0 halves of x and
        # the q/k weights first so the m=0 projections start ASAP
        XT = [[None, None], [None, None]]   # [kc][ic] -> [128, 512]
        WQC = [[None, None], [None, None], [None, None]]  # [col][kc]
        WOT = []

        def load_x(kc, ic, eng):
            t = wpool.tile([128, 512], F16, tag=f"xt{kc}{ic}", name=f"xt{kc}{ic}")
            eng.dma_start(
                t[:], xt[kc * 128 : (kc + 1) * 128, ic * 512 : (ic + 1) * 512]
            )
            XT[kc][ic] = t

        def load_w(col, kc, eng):
            w = wpool.tile([128, HD], F16, tag=f"w{col}{kc}", name=f"w{col}{kc}")
            eng.dma_start(
                w[:], wqkv[kc * 128 : (kc + 1) * 128, col * HD : (col + 1) * HD]
            )
            WQC[col][kc] = w

        # ALL input loads go on the SP HWDGE queue, ordered by first use.
        # (The ACT queue carries the output DMAs; mixing input loads onto it
        # would park the next loop iteration's weights behind this
        # iteration's output tail -- measured ~9us/iter of PE idle.)
        load_x(0, 0, nc.sync); load_x(1, 0, nc.scalar)
        load_w(0, 0, nc.sync); load_w(0, 1, nc.scalar)
        load_x(0, 1, nc.sync); load_x(1, 1, nc.scalar)
        load_w(1, 0, nc.sync); load_w(1, 1, nc.scalar)
        load_w(2, 0, nc.sync); load_w(2, 1, nc.scalar)
        for kt in range(4):
            w = wpool.tile([128, O], F16, tag=f"wot{kt}", name=f"wot{kt}")
            nc.sync.dma_start(w[:], wot[kt * 128 : (kt + 1) * 128, :])
            WOT.append(w)

        QT = [qkvpool.tile([128, N], F16, tag=f"q{m}", name=f"q{m}") for m in range(4)]
        KT = [qkvpool.tile([128, N], F16, tag=f"k{m}", name=f"k{m}") for m in range(4)]
        VT = [qkvpool.tile([128, N], F16, tag=f"v{m}", name=f"v{m}") for m in range(4)]
        APP = [qkvpool.tile([128, N], F16, tag=f"app{m}", name=f"app{m}") for m in range(4)]

        with (
            tc.tile_pool(name="sps", bufs=2, space="PSUM") as sps,
            tc.tile_pool(name="pps", bufs=1, space="PSUM") as pps,
            tc.tile_pool(name="awps", bufs=1, space="PSUM") as awps,
        ):

            def project(col, m, dst):
                """dst[hd', i] = sum_c W[c, col*HD + m*128 + hd'] * xT[c, i]"""
                ps = pps.tile([128, N], FP32, tag="p")
                for ic in range(2):
                    for kc in range(2):
                        nc.tensor.matmul(
                            ps[:, ic * 512 : (ic + 1) * 512],
                            WQC[col][kc][:, m * 128 : (m + 1) * 128],
                            XT[kc][ic][:],
                            start=(kc == 0),
                            stop=(kc == 1),
                        )
                with nc.allow_low_precision(reason="f16 activations"):
                    nc.vector.tensor_copy(dst[:], ps[:])

            def project_t(t):
                project(0, t, QT[t])
                project(1, t, KT[t])
                project(2, t, VT[t])

            project_t(0)

            z_idx = 0
            for t in range(4):  # head pair (2t, 2t+1)
                # zab/izab column pairs (2jt, 2jt+1) hold the a/b halves
                zab = zpool.tile([128, 16], FP32, tag="zab")
                izab = izpool.tile([128, 16], F16, tag="izab")
                # one [128, 2048] tile per jt: cols 0-1023 head 2t (a),
                # 1024-2047 head 2t+1 (b) -- lets Z reduce both in one op
                G = [gpool.tile([128, 2 * N], F16, tag=f"g{jt}", name=f"g{jt}")
                     for jt in range(8)]
                aw = awps.tile([128, N], FP32, tag="aw")

                def aw_mm(jt):
                    """attnw colsum for tile jt (bcast-over-d lhsT, 2 heads
                    col-packed, accumulated over all jt in one PSUM group)."""
                    for ic in range(2):
                        icsl = slice(ic * 512, (ic + 1) * 512)
                        bicsl = slice(N + ic * 512, N + (ic + 1) * 512)
                        nc.tensor.matmul(
                            aw[0:64, icsl], _bcast64(izab[:, 2 * jt : 2 * jt + 1]),
                            G[jt][:, icsl],
                            start=(jt == 0), stop=(jt == 7),
                            tile_position=(0, 0), skip_group_check=True,
                        )
                        nc.tensor.matmul(
                            aw[64:128, icsl], _bcast64(izab[:, 2 * jt + 1 : 2 * jt + 2]),
                            G[jt][:, bicsl],
                            start=(jt == 0), stop=(jt == 7),
                            tile_position=(0, 64), skip_group_check=True,
                        )

                for jt in range(8):
                    jsl = slice(jt * 128, (jt + 1) * 128)
                    sa = sps.tile([128, N], FP32, tag="s")
                    sb_ = sps.tile([128, N], FP32, tag="s")
                    for ic in range(2):
                        icsl = slice(ic * 512, (ic + 1) * 512)
                        # two K=64 matmuls row-packed in the PE array
                        nc.tensor.matmul(
                            sa[:, icsl], KT[t][0:64, jsl], QT[t][0:64, icsl],
                            start=True, stop=True,
                        )
                        nc.tensor.matmul(
                            sb_[:, icsl], KT[t][64:128, jsl], QT[t][64:128, icsl],
                            start=True, stop=True, tile_position=(64, 0),
                        )
                    # Z row-sums: split between ACT accum_out (+284ns each
                    # on the bottleneck ACT) and one DVE free-axis reduce
                    # per jt covering both halves (~2.2us on DVE)
                    z_on_act = (z_idx * ZACC_NUM) % 32 < ZACC_NUM
                    z_idx += 1
                    for half, sps_tile in ((0, sa), (1, sb_)):
                        zc = slice(2 * jt + half, 2 * jt + half + 1)
                        nc.scalar.activation(
                            G[jt][:, half * N : (half + 1) * N], sps_tile[:],
                            EXP, scale=0.125,
                            accum_out=zab[:, zc] if z_on_act else None,
                        )
                    if not z_on_act:
                        nc.vector.tensor_reduce(
                            zab[:, 2 * jt : 2 * jt + 2], _split2(G[jt][:], N),
                            op=mybir.AluOpType.add,
                            axis=mybir.AxisListType.X,
                        )
                    with nc.allow_low_precision(reason="f16 matmul operands"):
                        nc.vector.reciprocal(
                            izab[:, 2 * jt : 2 * jt + 2],
                            zab[:, 2 * jt : 2 * jt + 2],
                        )
                    # attnw matmul for the previous jt keeps the PE warm
                    # while this jt's exps drain on ACT
                    if jt > 0:
                        aw_mm(jt - 1)
                    # spread the next head-pair's projections across this
                    # pair's score loop (PE/DVE work independent of ACT)
                    if t < 3 and jt in (2, 4, 6):
                        col = jt // 2 - 1
                        project(col, t + 1, (QT, KT, VT)[col][t + 1])
                aw_mm(7)
                with nc.allow_low_precision(reason="f16 activations"):
                    nc.vector.tensor_mul(APP[t][:], VT[t][:], aw[:])

        with tc.tile_pool(name="ops", bufs=2, space="PSUM") as ops:
            for it in range(8):
                itsl = slice(it * 128, (it + 1) * 128)
                po = ops.tile([128, O], FP32, tag="o")
                for kt in range(4):
                    nc.tensor.matmul(
                        po[:], APP[kt][:, itsl], WOT[kt][:],
                        start=(kt == 0), stop=(kt == 3),
                    )
                ob = obpool.tile([128, O], FP32, tag="ob")
                nc.vector.tensor_copy(ob[:], po[:])
                # out DMAs go on the ACT HWDGE queue: SP's in-order queue
                # must stay clear so the next loop iteration's input loads
                # prefetch during the output tail instead of behind it
                nc.scalar.dma_start(out[itsl, :], ob[:])


def build_nc(loop=0, use_bacc=False):
    cls = bacc.Bacc if use_bacc else bass.Bass
    nc = cls("TRN2", target_bir_lowering=False, debug=False, num_devices=N_CORES)
    xt = nc.declare_dram_parameter("xt", [C, N], F16, isOutput=False)
    wqkv = nc.declare_dram_parameter("wqkv", [C, 3 * HD], F16, isOutput=False)
    wot = nc.declare_dram_parameter("wot", [HD, O], F16, isOutput=False)
    out = nc.declare_dram_parameter("out", [N, O], FP32, isOutput=True)
    with _TC(nc, num_cores=N_CORES) as tc:
        # The For_i back-edge costs ~4.5us (all-engine barrier + semaphore
        # reset), so emit two bodies per hardware-loop iteration; an odd
        # remainder body runs outside the loop.
        if loop >= 2:
            with tc.For_i(0, loop // 2, 1):
                _emit_body(tc, xt.ap(), wqkv.ap(), wot.ap(), out.ap())
                _emit_body(tc, xt.ap(), wqkv.ap(), wot.ap(), out.ap())
        for _ in range(loop % 2 if loop else 1):
            _emit_body(tc, xt.ap(), wqkv.ap(), wot.ap(), out.ap())
    return nc


def make_in_maps(features, weight_q, weight_k, weight_v, weight_out):
    wqkv = np.ascontiguousarray(
        np.concatenate(
            [
                weight_q.reshape(C, HD),
                weight_k.reshape(C, HD),
                weight_v.reshape(C, HD),
            ],
            axis=1,
        ),
        dtype=np.float16,
    )
    wot = np.ascontiguousarray(weight_out.reshape(O, HD).T, dtype=np.float16)
    in_maps = []
    for b in range(B):
        xt = np.ascontiguousarray(features[b].T, dtype=np.float16)
        in_maps.append({"xt": xt, "wqkv": wqkv, "wot": wot})
    return in_maps


_CACHED_NC = None


def kernel(features, weight_q, weight_k, weight_v, weight_out):
    global _CACHED_NC
    if _CACHED_NC is None:
        _CACHED_NC = build_nc(loop=0)
    in_maps = make_in_maps(
        np.asarray(features, np.float32),
        np.asarray(weight_q, np.float32),
        np.asarray(weight_k, np.float32),
        np.asarray(weight_v, np.float32),
        np.asarray(weight_out, np.float32),
    )
    res = run_bass_kernel_spmd(_CACHED_NC, in_maps, list(range(N_CORES)))
    return np.stack([res.results[b]["out"] for b in range(B)], axis=0)


if __name__ == "__main__":
    rng = np.random.default_rng(0)
    feats = rng.standard_normal((B, N, C)).astype(np.float32)
    wq = rng.standard_normal((C, H, D)).astype(np.float32) * 0.05
    wk = rng.standard_normal((C, H, D)).astype(np.float32) * 0.05
    wv = rng.standard_normal((C, H, D)).astype(np.float32) * 0.05
    wo = rng.standard_normal((O, H, D)).astype(np.float32) * 0.05
    o = kernel(feats, wq, wk, wv, wo)
    print("kernel ran, out shape", o.shape, "finite:", np.isfinite(o).all())


# revision 25
# speedup vs baseline: 1.4371x; 1.4371x over previous
"""Trainium2 Bass kernel for nn_MultiHeadAttention_62371515073076.

Math (per batch b, faithful to the reference's quirky softmax over the QUERY axis):
  q/k/v = einsum('nc,chd->nhd', x, W{q,k,v})
  s[i,j,h] = q[i,h,:].k[j,h,:] / 8
  p = softmax over i  (query axis!)
  attnw[i,h] = sum_j p[i,j,h]
             = sum_j exp(s[i,j,h]) / Z[j,h],   Z[j,h] = sum_i exp(s[i,j,h])
  out = einsum('ihd,ohd->io', v * attnw, Wout)

Sharding: batch 8 -> one batch per NeuronCore (data parallel), weights replicated.

Per-core layout (all fp16 activations, fp32 psum/accum):
  - Host pre-transposes x -> xt (C,N) and Wout -> wot (HD,O); wq/wk/wv fed
    natural (C,HD) concatenated = the lhsT layout the PE wants.
  - QKV projections produce transposed Q^T/K^T/V^T [hd, i] tiles.
  - Scores computed transposed, S^T [j, i], per head, K=64 row-packed two
    heads per PE pass (partitions 0-63 / 64-127).
  - exp via ScalarE reading the score PSUM directly (HW-measured: ScalarE is
    1x rate regardless of dtype/space, so any SBUF bounce just wastes DVE).
    Z[j] row-sums are split between ScalarE accum_out (+282ns/tile on ACT)
    and gpsimd tensor_reduce on the otherwise-idle Pool engine.
  - attnw colsum matmuls INTERLEAVE with the score loop (lag 1 jt) so the PE
    never idles >3.4us -- HAM re-throttles the PE clock 2.4->1.2 GHz after
    ~3.4us idle, which was worth ~40us/iter. lhsT = 1/Z column broadcast via
    a step-0 AP (two heads col-packed, accumulated over j in PSUM).
  - Next head-pair's projections are spread across the score loop so the PE
    always has independent work queued.
  - applied^T = V^T * attnw_bcast (DVE), then output projection back to
    natural [i, o] layout and DMA out.
"""
import os
import numpy as np
from contextlib import ExitStack

import concourse.bass as bass
import concourse.mybir as mybir
import concourse.tile as tile
from concourse import bacc
from concourse.vector_clock import ScopedClock
from concourse.bass_utils import run_bass_kernel_spmd
import bass_rust

N_CORES = 8
B, N, C, H, D, O = 8, 1024, 256, 8, 64, 256
HD = H * D  # 512
FP32 = mybir.dt.float32
F16 = mybir.dt.float16
EXP = mybir.ActivationFunctionType.Exp

# Of every 32 per-jt Z row-sum pairs, this many use ScalarE accum_out; the
# rest run as one DVE free-axis reduce per jt. Balances ACT vs DVE.
ZACC_NUM = int(os.environ.get("ZACC_NUM", "10"))

_MAXW = 1  # max sync waits this toolchain's walrus accepts per instruction


class _TC(tile.TileContext):
    """TileContext that splits semaphore waits one-per-instruction.

    The walrus build in this toolchain rejects any instruction carrying more
    than one sync wait ("Too many sync wait commands"), while Tile's
    add_semaphores attaches all needed waits to the consuming instruction.
    Engines execute in order, so moving excess waits onto same-engine NOPs
    emitted immediately before the instruction is semantically identical.
    """

    def _commit_instruction(self, inst, lazy_reg_writes: bool = True):
        si = inst.sync_info
        if (
            si is not None
            and si.on_wait
            and len(si.on_wait) > _MAXW
            and inst.engine != mybir.EngineType.Unassigned
        ):
            waits = list(si.on_wait)
            inst.sync_info = bass_rust.SyncInfo(
                on_wait=waits[-_MAXW:], on_update=list(si.on_update or [])
            )
            for i in range(0, len(waits) - _MAXW, _MAXW):
                nop = self.nc.engines[inst.engine].nop(nofuse=True, hint="waitsplit")
                nop.ins.sync_info = bass_rust.SyncInfo(
                    on_wait=waits[i : i + _MAXW], on_update=[]
                )
        return super()._commit_instruction(inst, lazy_reg_writes)

    def _drain_and_barrier(self, tick_clock, wait_clock):
        probe = self.nc.sync.drain()
        wait_clock.add_sem_waits(
            probe.ins, ScopedClock({None: tick_clock.global_clock})
        )
        si = probe.ins.sync_info
        waits = list(si.on_wait or []) if si is not None else []
        if len(waits) > 1:
            probe.ins.sync_info = bass_rust.SyncInfo(
                on_wait=waits[:1], on_update=list(si.on_update or [])
            )
            for i in range(1, len(waits)):
                d = self.nc.sync.drain()
                d.ins.sync_info = bass_rust.SyncInfo(
                    on_wait=waits[i : i + 1], on_update=[]
                )
        self.nc.all_engine_barrier()
        assert self.sems is not None
        popped = self.nc._tile_sem_poison_stack.pop()
        assert popped is self._sem_poison
        self.nc.clear_and_free_semaphores(list(self.sems.allocated().values()))
        self.nc.all_engine_barrier()


def _bcast64(col_ap):
    """[P,1] AP -> [P,64] AP reading the same element 64x (free step 0)."""
    return bass.AP(col_ap.tensor, col_ap.offset, [list(col_ap.ap[0]), [0, 64]])


def _split2(ap, inner):
    """[P,2*inner] AP -> [P,2,inner] AP (for per-half free-axis reduces)."""
    return bass.AP(ap.tensor, ap.offset, [list(ap.ap[0]), [inner, 2], [1, inner]])


def _emit_body(tc, xt, wqkv, wot, out):
    nc = tc.nc
    with ExitStack() as ctx:
        wpool = ctx.enter_context(tc.tile_pool(name="w", bufs=1))
        qkvpool = ctx.enter_context(tc.tile_pool(name="qkv", bufs=1))
        gpool = ctx.enter_context(tc.tile_pool(name="g", bufs=2))
        izpool = ctx.enter_context(tc.tile_pool(name="iz", bufs=2))
        zpool = ctx.enter_context(tc.tile_pool(name="z", bufs=2))
        obpool = ctx.enter_context(tc.tile_pool(name="ob", bufs=2))

        # fine-grained input loads, ordered by first use: ic0 halves of x and
        # the q/k weights first so the m=0 projections start ASAP
        XT = [[None, None], [None, None]]   # [kc][ic] -> [128, 512]
        WQC = [[None, None], [None, None], [None, None]]  # [col][kc]
        WOT = []

        def load_x(kc, ic, eng):
            t = wpool.tile([128, 512], F16, tag=f"xt{kc}{ic}", name=f"xt{kc}{ic}")
            eng.dma_start(
                t[:], xt[kc * 128 : (kc + 1) * 128, ic * 512 : (ic + 1) * 512]
            )
            XT[kc][ic] = t

        def load_w(col, kc, eng):
            w = wpool.tile([128, HD], F16, tag=f"w{col}{kc}", name=f"w{col}{kc}")
            eng.dma_start(
                w[:], wqkv[kc * 128 : (kc + 1) * 128, col * HD : (col + 1) * HD]
            )
            WQC[col][kc] = w

        # ALL input loads go on the SP HWDGE queue, ordered by first use.
        # (The ACT queue carries the output DMAs; mixing input loads onto it
        # would park the next loop iteration's weights behind this
        # iteration's output tail -- measured ~9us/iter of PE idle.)
        load_x(0, 0, nc.sync); load_x(1, 0, nc.scalar)
        load_w(0, 0, nc.sync); load_w(0, 1, nc.scalar)
        load_x(0, 1, nc.sync); load_x(1, 1, nc.scalar)
        load_w(1, 0, nc.sync); load_w(1, 1, nc.scalar)
        load_w(2, 0, nc.sync); load_w(2, 1, nc.scalar)
        for kt in range(4):
            w = wpool.tile([128, O], F16, tag=f"wot{kt}", name=f"wot{kt}")
            nc.sync.dma_start(w[:], wot[kt * 128 : (kt + 1) * 128, :])
            WOT.append(w)

        QT = [qkvpool.tile([128, N], F16, tag=f"q{m}", name=f"q{m}") for m in range(4)]
        KT = [qkvpool.tile([128, N], F16, tag=f"k{m}", name=f"k{m}") for m in range(4)]
        VT = [qkvpool.tile([128, N], F16, tag=f"v{m}", name=f"v{m}") for m in range(4)]
        APP = [qkvpool.tile([128, N], F16, tag=f"app{m}", name=f"app{m}") for m in range(4)]

        with (
            tc.tile_pool(name="sps", bufs=2, space="PSUM") as sps,
            tc.tile_pool(name="pps", bufs=1, space="PSUM") as pps,
            tc.tile_pool(name="awps", bufs=1, space="PSUM") as awps,
        ):

            def project(col, m, dst):
                """dst[hd', i] = sum_c W[c, col*HD + m*128 + hd'] * xT[c, i]"""
                ps = pps.tile([128, N], FP32, tag="p")
                for ic in range(2):
                    for kc in range(2):
                        nc.tensor.matmul(
                            ps[:, ic * 512 : (ic + 1) * 512],
                            WQC[col][kc][:, m * 128 : (m + 1) * 128],
                            XT[kc][ic][:],
                            start=(kc == 0),
                            stop=(kc == 1),
                        )
                with nc.allow_low_precision(reason="f16 activations"):
                    nc.vector.tensor_copy(dst[:], ps[:])

            def project_t(t):
                project(0, t, QT[t])
                project(1, t, KT[t])
                project(2, t, VT[t])

            project_t(0)

            z_idx = 0
            for t in range(4):  # head pair (2t, 2t+1)
                # zab/izab column pairs (2jt, 2jt+1) hold the a/b halves
                zab = zpool.tile([128, 16], FP32, tag="zab")
                izab = izpool.tile([128, 16], F16, tag="izab")
                # one [128, 2048] tile per jt: cols 0-1023 head 2t (a),
                # 1024-2047 head 2t+1 (b) -- lets Z reduce both in one op
                G = [gpool.tile([128, 2 * N], F16, tag=f"g{jt}", name=f"g{jt}")
                     for jt in range(8)]
                aw = awps.tile([128, N], FP32, tag="aw")

                def aw_mm(jt):
                    """attnw colsum for tile jt (bcast-over-d lhsT, 2 heads
                    col-packed, accumulated over all jt in one PSUM group)."""
                    for ic in range(2):
                        icsl = slice(ic * 512, (ic + 1) * 512)
                        bicsl = slice(N + ic * 512, N + (ic + 1) * 512)
                        nc.tensor.matmul(
                            aw[0:64, icsl], _bcast64(izab[:, 2 * jt : 2 * jt + 1]),
                            G[jt][:, icsl],
                            start=(jt == 0), stop=(jt == 7),
                            tile_position=(0, 0), skip_group_check=True,
                        )
                        nc.tensor.matmul(
                            aw[64:128, icsl], _bcast64(izab[:, 2 * jt + 1 : 2 * jt + 2]),
                            G[jt][:, bicsl],
                            start=(jt == 0), stop=(jt == 7),
                            tile_position=(0, 64), skip_group_check=True,
                        )

                for jt in range(8):
                    jsl = slice(jt * 128, (jt + 1) * 128)
                    sa = sps.tile([128, N], FP32, tag="s")
                    sb_ = sps.tile([128, N], FP32, tag="s")
                    for ic in range(2):
                        icsl = slice(ic * 512, (ic + 1) * 512)
                        # two K=64 matmuls row-packed in the PE array
                        nc.tensor.matmul(
                            sa[:, icsl], KT[t][0:64, jsl], QT[t][0:64, icsl],
                            start=True, stop=True,
                        )
                        nc.tensor.matmul(
                            sb_[:, icsl], KT[t][64:128, jsl], QT[t][64:128, icsl],
                            start=True, stop=True, tile_position=(64, 0),
                        )
                    # Z row-sums: split between ACT accum_out (+284ns each
                    # on the bottleneck ACT) and one DVE free-axis reduce
                    # per jt covering both halves (~2.2us on DVE)
                    z_on_act = (z_idx * ZACC_NUM) % 32 < ZACC_NUM
                    z_idx += 1
                    for half, sps_tile in ((0, sa), (1, sb_)):
                        zc = slice(2 * jt + half, 2 * jt + half + 1)
                        nc.scalar.activation(
                            G[jt][:, half * N : (half + 1) * N], sps_tile[:],
                            EXP, scale=0.125,
                            accum_out=zab[:, zc] if z_on_act else None,
                        )
                    if not z_on_act:
                        nc.vector.tensor_reduce(
                            zab[:, 2 * jt : 2 * jt + 2], _split2(G[jt][:], N),
                            op=mybir.AluOpType.add,
                            axis=mybir.AxisListType.X,
                        )
                    with nc.allow_low_precision(reason="f16 matmul operands"):
                        nc.vector.reciprocal(
                            izab[:, 2 * jt : 2 * jt + 2],
                            zab[:, 2 * jt : 2 * jt + 2],
                        )
                    # attnw matmul for the previous jt keeps the PE warm
                    # while this jt's exps drain on ACT
                    if jt > 0:
                        aw_mm(jt - 1)
                    # spread the next head-pair's projections across this
                    # pair's score loop (PE/DVE work independent of ACT)
                    if t < 3 and jt in (2, 4, 6):
                        col = jt // 2 - 1
                        project(col, t + 1, (QT, KT, VT)[col][t + 1])
                aw_mm(7)
                with nc.allow_low_precision(reason="f16 activations"):
                    nc.vector.tensor_mul(APP[t][:], VT[t][:], aw[:])

        with tc.tile_pool(name="ops", bufs=2, space="PSUM") as ops:
            for it in range(8):
                itsl = slice(it * 128, (it + 1) * 128)
                po = ops.tile([128, O], FP32, tag="o")
                for kt in range(4):
                    nc.tensor.matmul(
                        po[:], APP[kt][:, itsl], WOT[kt][:],
                        start=(kt == 0), stop=(kt == 3),
                    )
                ob = obpool.tile([128, O], FP32, tag="ob")
                nc.vector.tensor_copy(ob[:], po[:])
                # out DMAs go on the ACT HWDGE queue: SP's in-order queue
                # must stay clear so the next loop iteration's input loads
                # prefetch during the output tail instead of behind it
                nc.scalar.dma_start(out[itsl, :], ob[:])


def build_nc(loop=0, use_bacc=False):
    cls = bacc.Bacc if use_bacc else bass.Bass
    nc = cls("TRN2", target_bir_lowering=False, debug=False, num_devices=N_CORES)
    xt = nc.declare_dram_parameter("xt", [C, N], F16, isOutput=False)
    wqkv = nc.declare_dram_parameter("wqkv", [C, 3 * HD], F16, isOutput=False)
    wot = nc.declare_dram_parameter("wot", [HD, O], F16, isOutput=False)
    out = nc.declare_dram_parameter("out", [N, O], FP32, isOutput=True)
    with _TC(nc, num_cores=N_CORES) as tc:
        if loop:
            with tc.For_i(0, loop, 1):
                _emit_body(tc, xt.ap(), wqkv.ap(), wot.ap(), out.ap())
        else:
            _emit_body(tc, xt.ap(), wqkv.ap(), wot.ap(), out.ap())
    return nc


def make_in_maps(features, weight_q, weight_k, weight_v, weight_out):
    wqkv = np.ascontiguousarray(
        np.concatenate(
            [
                weight_q.reshape(C, HD),
                weight_k.reshape(C, HD),
                weight_v.reshape(C, HD),
            ],
            axis=1,
        ),
        dtype=np.float16,
    )
    wot = np.ascontiguousarray(weight_out.reshape(O, HD).T, dtype=np.float16)
    in_maps = []
    for b in range(B):
        xt = np.ascontiguousarray(features[b].T, dtype=np.float16)
        in_maps.append({"xt": xt, "wqkv": wqkv, "wot": wot})
    return in_maps


_CACHED_NC = None


def kernel(features, weight_q, weight_k, weight_v, weight_out):
    global _CACHED_NC
    if _CACHED_NC is None:
        _CACHED_NC = build_nc(loop=0)
    in_maps = make_in_maps(
        np.asarray(features, np.float32),
        np.asarray(weight_q, np.float32),
        np.asarray(weight_k, np.float32),
        np.asarray(weight_v, np.float32),
        np.asarray(weight_out, np.float32),
    )
    res = run_bass_kernel_spmd(_CACHED_NC, in_maps, list(range(N_CORES)))
    return np.stack([res.results[b]["out"] for b in range(B)], axis=0)


if __name__ == "__main__":
    rng = np.random.default_rng(0)
    feats = rng.standard_normal((B, N, C)).astype(np.float32)
    wq = rng.standard_normal((C, H, D)).astype(np.float32) * 0.05
    wk = rng.standard_normal((C, H, D)).astype(np.float32) * 0.05
    wv = rng.standard_normal((C, H, D)).astype(np.float32) * 0.05
    wo = rng.standard_normal((O, H, D)).astype(np.float32) * 0.05
    o = kernel(feats, wq, wk, wv, wo)
    print("kernel ran, out shape", o.shape, "finite:", np.isfinite(o).all())
